# revision 4
# baseline (speedup 1.0000x reference)
"""Trainium2 Bass kernel for nn_AllAmplitude (helicity-amplitude intensity).

Math: the reference contracts two spin-1 Wigner-D matrices per (resonance,
event) with a Breit-Wigner weight and sums |amp|^2 over external helicities.
Because D1 @ D2 = D^1(U1 U2) for the SU(2) elements U1, U2 of the two
rotations, and sum_{a,dlt} mult_dlt M_r M*_r' = 2 tr D(V) + D(V)[0,0] with
V = U_r'^dag U_r, the whole intensity collapses to

  I = 7 sum_r |w_r|^2 + sum_{r<r'} 2 Re(w_r w*_r') (10 Re(av)^2 + 2 Im(av)^2 - 3)

with av = conj(a_r') a_r + b_r' conj(b_r), (a,b) the Cayley-Klein parameters
of the composed rotation, w_r the complex BW weight.  Per event this is ~200
flops instead of the reference's ~3000.

Sharding: pure data parallelism; the event axis N=262144 is split across the
8 NeuronCores (32768 events each, laid out as [128 partitions x 256 events],
with the R=4 resonance slices side by side in the free axis -> [128, 1024]
working tiles).
"""

import numpy as np
from contextlib import ExitStack

import concourse.bass as bass
import concourse.tile as tile
from concourse import bacc, mybir
from concourse.bass_utils import run_bass_kernel_spmd

F32 = mybir.dt.float32
ALU = mybir.AluOpType
ACTF = mybir.ActivationFunctionType

R = 4
N_TOTAL = 262144
N_CORES = 8
N_CORE = N_TOTAL // N_CORES     # 32768 events per core
P = 128                         # SBUF partitions
E = N_CORE // P                 # 256 events per partition per resonance
W = R * E                       # 1024 free-dim of a full working tile

MAGIC = float(np.float32(1.5 * 2.0**23))   # round-to-nearest-int bias trick
INV4PI = float(np.float32(1.0 / (4.0 * np.pi)))
TWOPI = float(np.float32(2.0 * np.pi))
HALFPI = float(np.float32(np.pi / 2.0))

INPUT_NAMES = ("alpha1", "beta1", "gamma1", "alpha2", "beta2", "gamma2", "m")


def _rs(r):
    return slice(r * E, (r + 1) * E)


def _build(m0, g0, coef_r, coef_i):
    """Build + compile the single-core graph (SPMD across 8 cores)."""
    nc = bacc.Bacc("TRN2", target_bir_lowering=False, debug=False,
                   num_devices=N_CORES)

    ins = {k: nc.dram_tensor(k, (R, N_CORE), F32, kind="ExternalInput").ap()
           for k in INPUT_NAMES}
    out_ap = nc.dram_tensor("out", (N_CORE,), F32, kind="ExternalOutput").ap()

    # per-resonance scalar constants (folded into instructions)
    m0 = m0.astype(np.float64); g0 = g0.astype(np.float64)
    f32 = np.float32
    cR = [float(f32(coef_r[r] * np.cos(coef_i[r]))) for r in range(R)]
    cI = [float(f32(coef_r[r] * np.sin(coef_i[r]))) for r in range(R)]
    m0sq = [float(f32(m0[r] * m0[r])) for r in range(R)]
    y = [float(f32(m0[r] * g0[r])) for r in range(R)]
    ysq = [float(f32(f32(y[r]) * f32(y[r]))) for r in range(R)]
    k1 = [float(f32(-f32(cI[r]) * f32(y[r]))) for r in range(R)]
    k2 = [float(f32(f32(cR[r]) * f32(y[r]))) for r in range(R)]

    with tile.TileContext(nc) as tc, ExitStack() as ctx:
        pin = ctx.enter_context(tc.tile_pool(name="pin", bufs=1))
        keep = ctx.enter_context(tc.tile_pool(name="keep", bufs=1))
        rot = ctx.enter_context(tc.tile_pool(name="rot", bufs=1))
        cnt = [0]

        def rtile(tag, bufs, shape=None):
            cnt[0] += 1
            return rot.tile(shape or [P, W], F32, tag=tag, bufs=bufs,
                            name=f"t{cnt[0]}")

        base = lambda: rtile("base", 4)
        chain = lambda: rtile("chain", 6)
        trig = lambda: rtile("trig", 12)
        pq = lambda: rtile("pq", 5)
        prot = lambda n: rtile("prot", 9, [P, n])
        small = lambda: rtile("small", 4, [P, E])

        # ---- DMA inputs (compute-critical angle tensors first) ----
        t_in = {}
        for k in ("alpha1", "alpha2", "gamma1", "gamma2", "beta1", "beta2", "m"):
            t = pin.tile([P, W], F32, tag=f"in_{k}", name=f"in_{k}")
            for r in range(R):
                nc.sync.dma_start(t[:, _rs(r)],
                                  ins[k][r].rearrange("(p e) -> p e", p=P, e=E))
            t_in[k] = t

        pi2 = keep.tile([P, 1], F32, tag="pi2", name="pi2")
        nc.vector.memset(pi2[:], HALFPI)

        # ---- composite angles ----
        # u=a1+a2 v=a1-a2 w=g1+g2 z=g1-g2 ; A=u+w B=v-z C=u+z D=v-w
        u = base(); nc.vector.tensor_add(u[:], t_in["alpha1"][:], t_in["alpha2"][:])
        v = base(); nc.vector.tensor_sub(v[:], t_in["alpha1"][:], t_in["alpha2"][:])
        w = base(); nc.vector.tensor_add(w[:], t_in["gamma1"][:], t_in["gamma2"][:])
        z = base(); nc.vector.tensor_sub(z[:], t_in["gamma1"][:], t_in["gamma2"][:])

        # ---- beta half-angle trig + Wigner-d magnitude products ----
        cb1 = trig(); nc.scalar.activation(cb1[:], t_in["beta1"][:], ACTF.Sin, scale=0.5, bias=pi2[:])
        sb1 = trig(); nc.scalar.activation(sb1[:], t_in["beta1"][:], ACTF.Sin, scale=0.5)
        cb2 = trig(); nc.scalar.activation(cb2[:], t_in["beta2"][:], ACTF.Sin, scale=0.5, bias=pi2[:])
        sb2 = trig(); nc.scalar.activation(sb2[:], t_in["beta2"][:], ACTF.Sin, scale=0.5)
        M1 = trig(); nc.vector.tensor_mul(M1[:], cb1[:], cb2[:])
        M2 = trig(); nc.vector.tensor_mul(M2[:], sb1[:], sb2[:])
        M3 = trig(); nc.vector.tensor_mul(M3[:], cb1[:], sb2[:])
        M4 = trig(); nc.vector.tensor_mul(M4[:], sb1[:], cb2[:])

        # For each half-angle X/2: t = X/(4pi)+off ; f = t-round(t) in
        # [-.5,.5] ; Sin(2pi f).  sin off 0.0 (A,B) / 0.5 (C,D -> negated,
        # cancels in av) ; cos off 0.25 (positive).  Immediately multiply by
        # the M magnitude so cs/sn tiles free quickly.
        def angle_products(Xa, Xb, add_op, s_off, M):
            X = chain()
            nc.vector.tensor_tensor(X[:], Xa[:], Xb[:], add_op)
            outs = []
            for off in (s_off, 0.25):
                t = chain()
                if off:
                    nc.vector.tensor_scalar(t[:], X[:], INV4PI, off, ALU.mult, ALU.add)
                else:
                    nc.vector.tensor_scalar(t[:], X[:], INV4PI, None, ALU.mult)
                rr = chain()
                nc.vector.tensor_scalar(rr[:], t[:], MAGIC, MAGIC, ALU.add, ALU.subtract)
                f = chain()
                nc.vector.tensor_sub(f[:], t[:], rr[:])
                sc = trig()
                nc.scalar.activation(sc[:], f[:], ACTF.Sin, scale=TWOPI)
                prod = pq()
                nc.vector.tensor_mul(prod[:], M[:], sc[:])
                outs.append(prod)
            return outs  # [M*sin-ish, M*cos]

        pa_s, pa_c = angle_products(u, w, ALU.add, 0.0, M1)       # M1 snA, M1 cA
        pb_s, pb_c = angle_products(v, z, ALU.subtract, 0.0, M2)  # M2 snB, M2 cB
        are = keep.tile([P, W], F32, tag="are", name="are")
        nc.vector.tensor_sub(are[:], pa_c[:], pb_c[:])
        aim = keep.tile([P, W], F32, tag="aim", name="aim")
        nc.vector.tensor_sub(aim[:], pb_s[:], pa_s[:])
        pc_s, pc_c = angle_products(u, z, ALU.add, 0.5, M3)       # -M3 snC, M3 cC
        pd_s, pd_c = angle_products(v, w, ALU.subtract, 0.5, M4)  # -M4 snD, M4 cD
        bre = keep.tile([P, W], F32, tag="bre", name="bre")
        nc.vector.tensor_add(bre[:], pc_c[:], pd_c[:])
        bim = keep.tile([P, W], F32, tag="bim", name="bim")
        nc.vector.tensor_add(bim[:], pc_s[:], pd_s[:])

        # ---- Breit-Wigner weights w_r = coef_r/(m0^2 - m^2 - i m0 g0) ----
        msq = chain(); nc.vector.tensor_mul(msq[:], t_in["m"][:], t_in["m"][:])
        x = chain()
        for r in range(R):
            nc.vector.tensor_scalar(x[:, _rs(r)], msq[:, _rs(r)], -1.0, m0sq[r],
                                    ALU.mult, ALU.add)
        xsq = chain(); nc.vector.tensor_mul(xsq[:], x[:], x[:])
        den = chain()
        for r in range(R):
            nc.vector.tensor_scalar(den[:, _rs(r)], xsq[:, _rs(r)], ysq[r], None, ALU.add)
        rc = chain()
        nc.vector.reciprocal_approx_fast(out=rc[:], in_=den[:])
        wp1 = chain(); wp2 = chain()
        for r in range(R):
            nc.vector.tensor_scalar(wp1[:, _rs(r)], x[:, _rs(r)], cR[r], k1[r], ALU.mult, ALU.add)
            nc.vector.tensor_scalar(wp2[:, _rs(r)], x[:, _rs(r)], cI[r], k2[r], ALU.mult, ALU.add)
        wre = keep.tile([P, W], F32, tag="wre", name="wre")
        wim = keep.tile([P, W], F32, tag="wim", name="wim")
        nc.vector.tensor_mul(wre[:], wp1[:], rc[:])
        nc.vector.tensor_mul(wim[:], wp2[:], rc[:])

        # diag = sum_r |w_r|^2
        d1 = chain(); nc.vector.tensor_mul(d1[:], wre[:], wre[:])
        d2 = chain(); nc.vector.tensor_mul(d2[:], wim[:], wim[:])
        dall = chain(); nc.vector.tensor_add(dall[:], d1[:], d2[:])
        dh = rtile("dh", 1, [P, 2 * E])
        nc.vector.tensor_add(dh[:], dall[:, 0:2*E], dall[:, 2*E:4*E])
        acc = keep.tile([P, E], F32, tag="acc", name="acc")
        # acc starts as 7*diag
        dg = small()
        nc.vector.tensor_add(dg[:], dh[:, 0:E], dh[:, E:2*E])
        nc.vector.tensor_scalar(acc[:], dg[:], 7.0, None, ALU.mult)

        # ---- pair interference terms, grouped by r-shift ----
        for sig in (1, 2, 3):
            n = (R - sig) * E
            L = slice(0, n)
            Rr = slice(sig * E, sig * E + n)

            def tmul(a, b):
                o = prot(n)
                nc.vector.tensor_mul(o[:], a[:, L], b[:, Rr])
                return o

            p1 = tmul(are, are); p2 = tmul(aim, aim)
            p3 = tmul(bre, bre); p4 = tmul(bim, bim)
            sa_ = prot(n); nc.vector.tensor_add(sa_[:], p1[:], p2[:])
            sb_ = prot(n); nc.vector.tensor_add(sb_[:], p3[:], p4[:])
            avr = prot(n); nc.vector.tensor_add(avr[:], sa_[:], sb_[:])

            q1 = tmul(are, aim); q2 = tmul(aim, are)
            q3 = tmul(bim, bre); q4 = tmul(bre, bim)
            ia = prot(n); nc.vector.tensor_sub(ia[:], q1[:], q2[:])
            ib = prot(n); nc.vector.tensor_sub(ib[:], q3[:], q4[:])
            avi = prot(n); nc.vector.tensor_add(avi[:], ia[:], ib[:])

            u20 = prot(n); nc.vector.scalar_tensor_tensor(u20[:], avr[:], 20.0, avr[:], ALU.mult, ALU.mult)
            c4 = prot(n); nc.vector.scalar_tensor_tensor(c4[:], avi[:], 4.0, avi[:], ALU.mult, ALU.mult)
            chis = prot(n); nc.vector.tensor_add(chis[:], u20[:], c4[:])

            g1 = tmul(wre, wre); g2 = tmul(wim, wim)
            gw = prot(n); nc.vector.tensor_add(gw[:], g1[:], g2[:])

            term = prot(n)
            nc.vector.scalar_tensor_tensor(term[:], chis[:], -6.0, gw[:], ALU.add, ALU.mult)
            for blk in range(R - sig):
                nc.vector.tensor_add(acc[:], acc[:], term[:, blk*E:(blk+1)*E])

        nc.sync.dma_start(out_ap.rearrange("(p e) -> p e", p=P, e=E), acc[:])

    nc.compile()
    return nc


_CACHE = {}


def _get_nc(m0, g0, coef_r, coef_i):
    key = (m0.tobytes(), g0.tobytes(), coef_r.tobytes(), coef_i.tobytes())
    if key not in _CACHE:
        _CACHE[key] = _build(m0, g0, coef_r, coef_i)
    return _CACHE[key]


def kernel(alpha1, beta1, gamma1, alpha2, beta2, gamma2, m, m0, g0,
           coef_r, coef_i, _want_trace=False):
    nc = _get_nc(np.asarray(m0, np.float32), np.asarray(g0, np.float32),
                 np.asarray(coef_r, np.float32), np.asarray(coef_i, np.float32))
    full = {"alpha1": alpha1, "beta1": beta1, "gamma1": gamma1,
            "alpha2": alpha2, "beta2": beta2, "gamma2": gamma2, "m": m}
    in_maps = []
    for i in range(N_CORES):
        sl = slice(i * N_CORE, (i + 1) * N_CORE)
        in_maps.append({k: np.ascontiguousarray(np.asarray(v, np.float32)[:, sl])
                        for k, v in full.items()})
    res = run_bass_kernel_spmd(nc, in_maps, core_ids=list(range(N_CORES)),
                               trace=_want_trace)
    out = np.concatenate([res.results[i]["out"] for i in range(N_CORES)])
    if _want_trace:
        kernel._last_result = res
    return out.astype(np.float32)


# revision 5
# speedup vs baseline: 1.4678x; 1.4678x over previous
"""Trainium2 Bass kernel for nn_AllAmplitude (helicity-amplitude intensity).

Math: the reference contracts two spin-1 Wigner-D matrices per (resonance,
event) with a Breit-Wigner weight and sums |amp|^2 over external helicities.
Because D1 @ D2 = D^1(U1 U2) for the SU(2) elements U1, U2 of the two
rotations, and sum_{a,dlt} mult_dlt M_r M*_r' = 2 tr D(V) + D(V)[0,0] with
V = U_r'^dag U_r, the whole intensity collapses to

  I = 7 sum_r |w_r|^2 + sum_{r<r'} 2 Re(w_r w*_r') (10 Re(av)^2 + 2 Im(av)^2 - 3)

with av = conj(a_r') a_r + b_r' conj(b_r), (a,b) the Cayley-Klein parameters
of the composed rotation, w_r the complex BW weight.  Per event this is ~200
flops instead of the reference's ~3000.

Sharding: pure data parallelism; the event axis N=262144 is split across the
8 NeuronCores (32768 events each, laid out as [128 partitions x 256 events],
with the R=4 resonance slices side by side in the free axis -> [128, 1024]
working tiles).

Implementation notes:
 - sin/cos of the four composite half-angles via fractional-turn range
   reduction (round-to-nearest through the 1.5*2^23 float trick) feeding the
   ScalarE Sin spline; the whole reduction is one fused custom-DVE op.
 - chi = 20 Re^2 + 4 Im^2 - 6 is a second fused custom-DVE op.
 - the bounded amplitude algebra runs in bf16 (2x DVE tensor_tensor rate);
   angles and Breit-Wigner stay fp32.
"""

import numpy as np
from contextlib import ExitStack

import concourse.bass as bass
import concourse.tile as tile
from concourse import bacc, mybir
from concourse.bass_utils import run_bass_kernel_spmd

F32 = mybir.dt.float32
BF16 = mybir.dt.bfloat16
ALU = mybir.AluOpType
ACTF = mybir.ActivationFunctionType

R = 4
N_TOTAL = 262144
N_CORES = 8
N_CORE = N_TOTAL // N_CORES     # 32768 events per core
P = 128                         # SBUF partitions
E = N_CORE // P                 # 256 events per partition per resonance
W = R * E                       # 1024 free-dim of a full working tile

MAGIC = float(np.float32(1.5 * 2.0**23))   # round-to-nearest-int bias trick
INV4PI = float(np.float32(1.0 / (4.0 * np.pi)))
TWOPI = float(np.float32(2.0 * np.pi))
HALFPI = float(np.float32(np.pi / 2.0))

BF16_ALGEBRA = True   # bounded amplitude algebra in bf16 (2x DVE TT rate)

INPUT_NAMES = ("alpha1", "beta1", "gamma1", "alpha2", "beta2", "gamma2", "m")


# ---------------------------------------------------------------------------
# custom fused DVE ops
# ---------------------------------------------------------------------------

def _register_custom_ops():
    import concourse.dve_ops as dve_ops
    from concourse.dve_spec import Spec, Src0, Src1, C0, C1, C2, sq, lower, _has_src1
    from concourse.dve_uop import DveOpSpec
    from concourse.dve_ops import DveOp

    if any(op.name == "ANT_RANGE_RED_ADD" for op in dve_ops.OPS):
        return {op.name: op for op in dve_ops.OPS}

    def make_op(name, spec):
        shas = {}
        for ver in ("v3", "v4"):
            uops = lower(spec, ver=ver)
            shas[ver] = DveOpSpec(name=name, opcode=31, uops=uops,
                                  rd1_en=_has_src1(spec)).sha(ver)
        return DveOp(name, spec, subdim=False, uops_sha=shas)

    def _rr_ref(sgn):
        def ref(in0, in1, s0, s1, imm2):
            t = ((in0 + sgn * in1) * s0 + s1).astype(np.float32)
            r = ((t + imm2).astype(np.float32) - imm2).astype(np.float32)
            return (t - r).astype(np.float32)
        return ref

    u = (Src0 + Src1) * C0 + C1
    rr_add = make_op("ANT_RANGE_RED_ADD",
                     Spec(body=u - ((u + C2) - C2), reference=_rr_ref(1.0)))
    u2 = (Src0 - Src1) * C0 + C1
    rr_sub = make_op("ANT_RANGE_RED_SUB",
                     Spec(body=u2 - ((u2 + C2) - C2), reference=_rr_ref(-1.0)))
    chi = make_op("ANT_CHI", Spec(
        body=sq(Src0) * C0 + sq(Src1) * C1 + C2,
        reference=lambda in0, in1, s0, s1, imm2:
            (in0 * in0 * s0 + in1 * in1 * s1 + imm2).astype(np.float32)))

    for op in (rr_add, rr_sub, chi):
        dve_ops.OPS.append(op)
        dve_ops._SUB_OPCODE_FOR_NAME[op.name] = (
            dve_ops._CUSTOM_DVE_ROW_BASE + len(dve_ops.OPS) - 1)
        dve_ops.CUSTOM_DVE_SPECS[op.name] = op.spec
    assert max(dve_ops._SUB_OPCODE_FOR_NAME.values()) < 0x20
    return {op.name: op for op in dve_ops.OPS}


def _rs(r):
    return slice(r * E, (r + 1) * E)


def _build(m0, g0, coef_r, coef_i):
    """Build + compile the single-core graph (SPMD across 8 cores)."""
    OPS = _register_custom_ops()
    RR_ADD, RR_SUB, CHI = (OPS["ANT_RANGE_RED_ADD"], OPS["ANT_RANGE_RED_SUB"],
                           OPS["ANT_CHI"])
    AT = BF16 if BF16_ALGEBRA else F32   # algebra dtype

    nc = bacc.Bacc("TRN2", target_bir_lowering=False, debug=False,
                   num_devices=N_CORES)

    ins = {k: nc.dram_tensor(k, (R, N_CORE), F32, kind="ExternalInput").ap()
           for k in INPUT_NAMES}
    out_ap = nc.dram_tensor("out", (N_CORE,), F32, kind="ExternalOutput").ap()

    # per-resonance scalar constants (folded into instructions)
    m0 = m0.astype(np.float64); g0 = g0.astype(np.float64)
    f32 = np.float32
    cR = [float(f32(coef_r[r] * np.cos(coef_i[r]))) for r in range(R)]
    cI = [float(f32(coef_r[r] * np.sin(coef_i[r]))) for r in range(R)]
    m0sq = [float(f32(m0[r] * m0[r])) for r in range(R)]
    y = [float(f32(m0[r] * g0[r])) for r in range(R)]
    ysq = [float(f32(f32(y[r]) * f32(y[r]))) for r in range(R)]
    k1 = [float(f32(-f32(cI[r]) * f32(y[r]))) for r in range(R)]
    k2 = [float(f32(f32(cR[r]) * f32(y[r]))) for r in range(R)]

    with tile.TileContext(nc) as tc, ExitStack() as ctx:
        pin = ctx.enter_context(tc.tile_pool(name="pin", bufs=1))
        keep = ctx.enter_context(tc.tile_pool(name="keep", bufs=1))
        rot = ctx.enter_context(tc.tile_pool(name="rot", bufs=1))
        cnt = [0]

        def rtile(tag, bufs, shape=None, dt=F32):
            cnt[0] += 1
            return rot.tile(shape or [P, W], dt, tag=tag, bufs=bufs,
                            name=f"t{cnt[0]}")

        base = lambda: rtile("base", 4)
        chain = lambda: rtile("chain", 6)
        trig = lambda: rtile("trig", 12, dt=AT)
        pq = lambda: rtile("pq", 5, dt=AT)
        prot = lambda n, dt=F32: rtile("prot", 10, [P, n], dt=dt)
        small = lambda: rtile("small", 4, [P, E])

        # ---- DMA inputs (compute-critical angle tensors first) ----
        t_in = {}
        for k in ("alpha1", "alpha2", "gamma1", "gamma2", "beta1", "beta2", "m"):
            t = pin.tile([P, W], F32, tag=f"in_{k}", name=f"in_{k}")
            for r in range(R):
                nc.sync.dma_start(t[:, _rs(r)],
                                  ins[k][r].rearrange("(p e) -> p e", p=P, e=E))
            t_in[k] = t

        pi2 = keep.tile([P, 1], F32, tag="pi2", name="pi2")
        nc.vector.memset(pi2[:], HALFPI)

        # ---- beta half-angle trig + Wigner-d magnitude products ----
        cb1 = trig(); nc.scalar.activation(cb1[:], t_in["beta1"][:], ACTF.Sin, scale=0.5, bias=pi2[:])
        sb1 = trig(); nc.scalar.activation(sb1[:], t_in["beta1"][:], ACTF.Sin, scale=0.5)
        cb2 = trig(); nc.scalar.activation(cb2[:], t_in["beta2"][:], ACTF.Sin, scale=0.5, bias=pi2[:])
        sb2 = trig(); nc.scalar.activation(sb2[:], t_in["beta2"][:], ACTF.Sin, scale=0.5)
        M1 = trig(); nc.vector.tensor_mul(M1[:], cb1[:], cb2[:])
        M2 = trig(); nc.vector.tensor_mul(M2[:], sb1[:], sb2[:])
        M3 = trig(); nc.vector.tensor_mul(M3[:], cb1[:], sb2[:])
        M4 = trig(); nc.vector.tensor_mul(M4[:], sb1[:], cb2[:])

        # ---- composite half-angles ----
        # A=(a1+a2)+(g1+g2) B=(a1-a2)-(g1-g2) C=(a1+a2)+(g1-g2) D=(a1-a2)-(g1+g2)
        u = base(); nc.vector.tensor_add(u[:], t_in["alpha1"][:], t_in["alpha2"][:])
        v = base(); nc.vector.tensor_sub(v[:], t_in["alpha1"][:], t_in["alpha2"][:])
        w = base(); nc.vector.tensor_add(w[:], t_in["gamma1"][:], t_in["gamma2"][:])
        z = base(); nc.vector.tensor_sub(z[:], t_in["gamma1"][:], t_in["gamma2"][:])

        # Per half-angle X/2: f = frac-reduce(X/(4pi)+off) in [-.5,.5] in one
        # fused DVE op, then Sin(2pi f) on ScalarE, then multiply by the M
        # magnitude.  sin off 0.0 (A,B) / 0.5 (C,D -> negated, cancels in av);
        # cos off 0.25 (positive).
        def angle_products(Xa, Xb, rrop, s_off, M):
            outs = []
            for off in (s_off, 0.25):
                f = chain()
                nc.vector._custom_dve(rrop, out=f[:], in0=Xa[:], in1=Xb[:],
                                      s0=INV4PI, s1=off, imm2=MAGIC)
                sc = trig()
                nc.scalar.activation(sc[:], f[:], ACTF.Sin, scale=TWOPI)
                prod = pq()
                nc.vector.tensor_mul(prod[:], M[:], sc[:])
                outs.append(prod)
            return outs  # [M*sin-ish, M*cos]

        pa_s, pa_c = angle_products(u, w, RR_ADD, 0.0, M1)       # M1 snA, M1 cA
        pb_s, pb_c = angle_products(v, z, RR_SUB, 0.0, M2)       # M2 snB, M2 cB
        are = keep.tile([P, W], AT, tag="are", name="are")
        nc.vector.tensor_sub(are[:], pa_c[:], pb_c[:])
        aim = keep.tile([P, W], AT, tag="aim", name="aim")
        nc.vector.tensor_sub(aim[:], pb_s[:], pa_s[:])
        pc_s, pc_c = angle_products(u, z, RR_ADD, 0.5, M3)       # -M3 snC, M3 cC
        pd_s, pd_c = angle_products(v, w, RR_SUB, 0.5, M4)       # -M4 snD, M4 cD
        bre = keep.tile([P, W], AT, tag="bre", name="bre")
        nc.vector.tensor_add(bre[:], pc_c[:], pd_c[:])
        bim = keep.tile([P, W], AT, tag="bim", name="bim")
        nc.vector.tensor_add(bim[:], pc_s[:], pd_s[:])

        # ---- Breit-Wigner weights w_r = coef_r/(m0^2 - m^2 - i m0 g0) ----
        msq = chain(); nc.vector.tensor_mul(msq[:], t_in["m"][:], t_in["m"][:])
        x = chain()
        for r in range(R):
            nc.vector.tensor_scalar(x[:, _rs(r)], msq[:, _rs(r)], -1.0, m0sq[r],
                                    ALU.mult, ALU.add)
        xsq = chain(); nc.vector.tensor_mul(xsq[:], x[:], x[:])
        den = chain()
        for r in range(R):
            nc.vector.tensor_scalar(den[:, _rs(r)], xsq[:, _rs(r)], ysq[r], None, ALU.add)
        rc = chain()
        nc.vector.reciprocal_approx_fast(out=rc[:], in_=den[:])
        wp1 = chain(); wp2 = chain()
        for r in range(R):
            nc.vector.tensor_scalar(wp1[:, _rs(r)], x[:, _rs(r)], cR[r], k1[r], ALU.mult, ALU.add)
            nc.vector.tensor_scalar(wp2[:, _rs(r)], x[:, _rs(r)], cI[r], k2[r], ALU.mult, ALU.add)
        wre = keep.tile([P, W], F32, tag="wre", name="wre")
        wim = keep.tile([P, W], F32, tag="wim", name="wim")
        nc.vector.tensor_mul(wre[:], wp1[:], rc[:])
        nc.vector.tensor_mul(wim[:], wp2[:], rc[:])
        if BF16_ALGEBRA:
            wreh = keep.tile([P, W], BF16, tag="wreh", name="wreh")
            wimh = keep.tile([P, W], BF16, tag="wimh", name="wimh")
            nc.vector.tensor_copy(wreh[:], wre[:])
            nc.vector.tensor_copy(wimh[:], wim[:])
        else:
            wreh, wimh = wre, wim

        # diag = sum_r |w_r|^2 (fp32)
        d1 = chain(); nc.vector.tensor_mul(d1[:], wre[:], wre[:])
        d2 = chain(); nc.vector.tensor_mul(d2[:], wim[:], wim[:])
        dall = chain(); nc.vector.tensor_add(dall[:], d1[:], d2[:])
        dh = rtile("dh", 1, [P, 2 * E])
        nc.vector.tensor_add(dh[:], dall[:, 0:2*E], dall[:, 2*E:4*E])
        acc = keep.tile([P, E], F32, tag="acc", name="acc")
        dg = small()
        nc.vector.tensor_add(dg[:], dh[:, 0:E], dh[:, E:2*E])
        nc.vector.tensor_scalar(acc[:], dg[:], 7.0, None, ALU.mult)

        # ---- pair interference terms, grouped by r-shift ----
        for sig in (1, 2, 3):
            n = (R - sig) * E
            L = slice(0, n)
            Rr = slice(sig * E, sig * E + n)

            def tmul(a, b):
                o = prot(n, AT)
                nc.vector.tensor_mul(o[:], a[:, L], b[:, Rr])
                return o

            p1 = tmul(are, are); p2 = tmul(aim, aim)
            p3 = tmul(bre, bre); p4 = tmul(bim, bim)
            sa_ = prot(n, AT); nc.vector.tensor_add(sa_[:], p1[:], p2[:])
            sb_ = prot(n, AT); nc.vector.tensor_add(sb_[:], p3[:], p4[:])
            avr = prot(n, AT); nc.vector.tensor_add(avr[:], sa_[:], sb_[:])

            q1 = tmul(are, aim); q2 = tmul(aim, are)
            q3 = tmul(bim, bre); q4 = tmul(bre, bim)
            ia = prot(n, AT); nc.vector.tensor_sub(ia[:], q1[:], q2[:])
            ib = prot(n, AT); nc.vector.tensor_sub(ib[:], q3[:], q4[:])
            avi = prot(n, AT); nc.vector.tensor_add(avi[:], ia[:], ib[:])

            chis = prot(n, F32)
            nc.vector._custom_dve(CHI, out=chis[:], in0=avr[:], in1=avi[:],
                                  s0=20.0, s1=4.0, imm2=-6.0)

            g1 = tmul(wreh, wreh); g2 = tmul(wimh, wimh)
            gw = prot(n, F32); nc.vector.tensor_add(gw[:], g1[:], g2[:])

            term = prot(n, F32)
            nc.vector.tensor_mul(term[:], chis[:], gw[:])
            for blk in range(R - sig):
                nc.vector.tensor_add(acc[:], acc[:], term[:, blk*E:(blk+1)*E])

        nc.sync.dma_start(out_ap.rearrange("(p e) -> p e", p=P, e=E), acc[:])

    nc.compile()
    return nc


_CACHE = {}


def _get_nc(m0, g0, coef_r, coef_i):
    key = (m0.tobytes(), g0.tobytes(), coef_r.tobytes(), coef_i.tobytes())
    if key not in _CACHE:
        _CACHE[key] = _build(m0, g0, coef_r, coef_i)
    return _CACHE[key]


def kernel(alpha1, beta1, gamma1, alpha2, beta2, gamma2, m, m0, g0,
           coef_r, coef_i, _want_trace=False):
    nc = _get_nc(np.asarray(m0, np.float32), np.asarray(g0, np.float32),
                 np.asarray(coef_r, np.float32), np.asarray(coef_i, np.float32))
    full = {"alpha1": alpha1, "beta1": beta1, "gamma1": gamma1,
            "alpha2": alpha2, "beta2": beta2, "gamma2": gamma2, "m": m}
    in_maps = []
    for i in range(N_CORES):
        sl = slice(i * N_CORE, (i + 1) * N_CORE)
        in_maps.append({k: np.ascontiguousarray(np.asarray(v, np.float32)[:, sl])
                        for k, v in full.items()})
    res = run_bass_kernel_spmd(nc, in_maps, core_ids=list(range(N_CORES)),
                               trace=_want_trace)
    out = np.concatenate([res.results[i]["out"] for i in range(N_CORES)])
    if _want_trace:
        kernel._last_result = res
    return out.astype(np.float32)


# revision 6
# speedup vs baseline: 1.4723x; 1.0031x over previous
"""Trainium2 Bass kernel for nn_AllAmplitude (helicity-amplitude intensity).

Math: the reference contracts two spin-1 Wigner-D matrices per (resonance,
event) with a Breit-Wigner weight and sums |amp|^2 over external helicities.
Because D1 @ D2 = D^1(U1 U2) for the SU(2) elements U1, U2 of the two
rotations, and sum_{a,dlt} mult_dlt M_r M*_r' = 2 tr D(V) + D(V)[0,0] with
V = U_r'^dag U_r, the whole intensity collapses to

  I = 7 sum_r |w_r|^2 + sum_{r<r'} 2 Re(w_r w*_r') (10 Re(av)^2 + 2 Im(av)^2 - 3)

with av = conj(a_r') a_r + b_r' conj(b_r), (a,b) the Cayley-Klein parameters
of the composed rotation, w_r the complex BW weight.  Per event this is ~200
flops instead of the reference's ~3000.

Sharding: pure data parallelism; the event axis N=262144 is split across the
8 NeuronCores (32768 events each, laid out as [128 partitions x 256 events],
with the R=4 resonance slices side by side in the free axis -> [128, 1024]
working tiles).

Implementation notes:
 - sin/cos of the four composite half-angles via fractional-turn range
   reduction (round-to-nearest through the 1.5*2^23 float trick) feeding the
   ScalarE Sin spline; the whole reduction is one fused custom-DVE op.
 - chi = 20 Re^2 + 4 Im^2 - 6 is a second fused custom-DVE op.
 - the bounded amplitude algebra runs in bf16 (2x DVE tensor_tensor rate);
   angles and Breit-Wigner stay fp32.
"""

import numpy as np
from contextlib import ExitStack

import concourse.bass as bass
import concourse.tile as tile
from concourse import bacc, mybir
from concourse.bass_utils import run_bass_kernel_spmd

F32 = mybir.dt.float32
BF16 = mybir.dt.bfloat16
FP16 = mybir.dt.float16
ALU = mybir.AluOpType
ACTF = mybir.ActivationFunctionType

R = 4
N_TOTAL = 262144
N_CORES = 8
N_CORE = N_TOTAL // N_CORES     # 32768 events per core
P = 128                         # SBUF partitions
E = N_CORE // P                 # 256 events per partition per resonance
W = R * E                       # 1024 free-dim of a full working tile

MAGIC = float(np.float32(1.5 * 2.0**23))   # round-to-nearest-int bias trick
INV4PI = float(np.float32(1.0 / (4.0 * np.pi)))
TWOPI = float(np.float32(2.0 * np.pi))
HALFPI = float(np.float32(np.pi / 2.0))

BF16_ALGEBRA = True   # bounded amplitude algebra in bf16 (2x DVE TT rate)

INPUT_NAMES = ("alpha1", "beta1", "gamma1", "alpha2", "beta2", "gamma2", "m")


# ---------------------------------------------------------------------------
# custom fused DVE ops
# ---------------------------------------------------------------------------

def _register_custom_ops():
    import concourse.dve_ops as dve_ops
    from concourse.dve_spec import Spec, Src0, Src1, C0, C1, C2, sq, lower, _has_src1
    from concourse.dve_uop import DveOpSpec
    from concourse.dve_ops import DveOp

    if any(op.name == "ANT_RANGE_RED_ADD" for op in dve_ops.OPS):
        return {op.name: op for op in dve_ops.OPS}

    def make_op(name, spec):
        shas = {}
        for ver in ("v3", "v4"):
            uops = lower(spec, ver=ver)
            shas[ver] = DveOpSpec(name=name, opcode=31, uops=uops,
                                  rd1_en=_has_src1(spec)).sha(ver)
        return DveOp(name, spec, subdim=False, uops_sha=shas)

    def _rr_ref(sgn):
        def ref(in0, in1, s0, s1, imm2):
            t = ((in0 + sgn * in1) * s0 + s1).astype(np.float32)
            r = ((t + imm2).astype(np.float32) - imm2).astype(np.float32)
            return (t - r).astype(np.float32)
        return ref

    u = (Src0 + Src1) * C0 + C1
    rr_add = make_op("ANT_RANGE_RED_ADD",
                     Spec(body=u - ((u + C2) - C2), reference=_rr_ref(1.0)))
    u2 = (Src0 - Src1) * C0 + C1
    rr_sub = make_op("ANT_RANGE_RED_SUB",
                     Spec(body=u2 - ((u2 + C2) - C2), reference=_rr_ref(-1.0)))
    chi = make_op("ANT_CHI", Spec(
        body=sq(Src0) * C0 + sq(Src1) * C1 + C2,
        reference=lambda in0, in1, s0, s1, imm2:
            (in0 * in0 * s0 + in1 * in1 * s1 + imm2).astype(np.float32)))

    for op in (rr_add, rr_sub, chi):
        dve_ops.OPS.append(op)
        dve_ops._SUB_OPCODE_FOR_NAME[op.name] = (
            dve_ops._CUSTOM_DVE_ROW_BASE + len(dve_ops.OPS) - 1)
        dve_ops.CUSTOM_DVE_SPECS[op.name] = op.spec
    assert max(dve_ops._SUB_OPCODE_FOR_NAME.values()) < 0x20
    return {op.name: op for op in dve_ops.OPS}


def _rs(r):
    return slice(r * E, (r + 1) * E)


def _build(m0, g0, coef_r, coef_i):
    """Build + compile the single-core graph (SPMD across 8 cores)."""
    OPS = _register_custom_ops()
    RR_ADD, RR_SUB, CHI = (OPS["ANT_RANGE_RED_ADD"], OPS["ANT_RANGE_RED_SUB"],
                           OPS["ANT_CHI"])
    AT = FP16 if BF16_ALGEBRA else F32   # bounded-amplitude algebra dtype

    nc = bacc.Bacc("TRN2", target_bir_lowering=False, debug=False,
                   num_devices=N_CORES)

    ins = {k: nc.dram_tensor(k, (R, N_CORE), F32, kind="ExternalInput").ap()
           for k in INPUT_NAMES}
    out_ap = nc.dram_tensor("out", (N_CORE,), F32, kind="ExternalOutput").ap()

    # per-resonance scalar constants (folded into instructions)
    m0 = m0.astype(np.float64); g0 = g0.astype(np.float64)
    f32 = np.float32
    cR = [float(f32(coef_r[r] * np.cos(coef_i[r]))) for r in range(R)]
    cI = [float(f32(coef_r[r] * np.sin(coef_i[r]))) for r in range(R)]
    m0sq = [float(f32(m0[r] * m0[r])) for r in range(R)]
    y = [float(f32(m0[r] * g0[r])) for r in range(R)]
    ysq = [float(f32(f32(y[r]) * f32(y[r]))) for r in range(R)]
    k1 = [float(f32(-f32(cI[r]) * f32(y[r]))) for r in range(R)]
    k2 = [float(f32(f32(cR[r]) * f32(y[r]))) for r in range(R)]

    with tile.TileContext(nc) as tc, ExitStack() as ctx:
        pin = ctx.enter_context(tc.tile_pool(name="pin", bufs=1))
        keep = ctx.enter_context(tc.tile_pool(name="keep", bufs=1))
        rot = ctx.enter_context(tc.tile_pool(name="rot", bufs=1))
        cnt = [0]

        def rtile(tag, bufs, shape=None, dt=F32):
            cnt[0] += 1
            return rot.tile(shape or [P, W], dt, tag=tag, bufs=bufs,
                            name=f"t{cnt[0]}")

        base = lambda: rtile("base", 4)
        chain = lambda: rtile("chain", 6)
        trig = lambda: rtile("trig", 12, dt=AT)
        pq = lambda: rtile("pq", 5, dt=AT)
        prot = lambda n, dt=F32: rtile("prot", 10, [P, n], dt=dt)
        small = lambda: rtile("small", 4, [P, E])

        # ---- DMA inputs (compute-critical angle tensors first) ----
        t_in = {}
        for k in ("alpha1", "alpha2", "gamma1", "gamma2", "beta1", "beta2", "m"):
            t = pin.tile([P, W], F32, tag=f"in_{k}", name=f"in_{k}")
            for r in range(R):
                nc.sync.dma_start(t[:, _rs(r)],
                                  ins[k][r].rearrange("(p e) -> p e", p=P, e=E))
            t_in[k] = t

        pi2 = keep.tile([P, 1], F32, tag="pi2", name="pi2")
        nc.vector.memset(pi2[:], HALFPI)

        # ---- beta half-angle trig + Wigner-d magnitude products ----
        cb1 = trig(); nc.scalar.activation(cb1[:], t_in["beta1"][:], ACTF.Sin, scale=0.5, bias=pi2[:])
        sb1 = trig(); nc.scalar.activation(sb1[:], t_in["beta1"][:], ACTF.Sin, scale=0.5)
        cb2 = trig(); nc.scalar.activation(cb2[:], t_in["beta2"][:], ACTF.Sin, scale=0.5, bias=pi2[:])
        sb2 = trig(); nc.scalar.activation(sb2[:], t_in["beta2"][:], ACTF.Sin, scale=0.5)
        M1 = trig(); nc.vector.tensor_mul(M1[:], cb1[:], cb2[:])
        M2 = trig(); nc.vector.tensor_mul(M2[:], sb1[:], sb2[:])
        M3 = trig(); nc.vector.tensor_mul(M3[:], cb1[:], sb2[:])
        M4 = trig(); nc.vector.tensor_mul(M4[:], sb1[:], cb2[:])

        # ---- composite half-angles ----
        # A=(a1+a2)+(g1+g2) B=(a1-a2)-(g1-g2) C=(a1+a2)+(g1-g2) D=(a1-a2)-(g1+g2)
        u = base(); nc.vector.tensor_add(u[:], t_in["alpha1"][:], t_in["alpha2"][:])
        v = base(); nc.vector.tensor_sub(v[:], t_in["alpha1"][:], t_in["alpha2"][:])
        w = base(); nc.vector.tensor_add(w[:], t_in["gamma1"][:], t_in["gamma2"][:])
        z = base(); nc.vector.tensor_sub(z[:], t_in["gamma1"][:], t_in["gamma2"][:])

        # Per half-angle X/2: f = frac-reduce(X/(4pi)+off) in [-.5,.5] in one
        # fused DVE op, then Sin(2pi f) on ScalarE, then multiply by the M
        # magnitude.  sin off 0.0 (A,B) / 0.5 (C,D -> negated, cancels in av);
        # cos off 0.25 (positive).
        def angle_products(Xa, Xb, rrop, s_off, M):
            outs = []
            for off in (s_off, 0.25):
                f = chain()
                nc.vector._custom_dve(rrop, out=f[:], in0=Xa[:], in1=Xb[:],
                                      s0=INV4PI, s1=off, imm2=MAGIC)
                sc = trig()
                nc.scalar.activation(sc[:], f[:], ACTF.Sin, scale=TWOPI)
                prod = pq()
                nc.vector.tensor_mul(prod[:], M[:], sc[:])
                outs.append(prod)
            return outs  # [M*sin-ish, M*cos]

        pa_s, pa_c = angle_products(u, w, RR_ADD, 0.0, M1)       # M1 snA, M1 cA
        pb_s, pb_c = angle_products(v, z, RR_SUB, 0.0, M2)       # M2 snB, M2 cB
        are = keep.tile([P, W], AT, tag="are", name="are")
        nc.vector.tensor_sub(are[:], pa_c[:], pb_c[:])
        aim = keep.tile([P, W], AT, tag="aim", name="aim")
        nc.vector.tensor_sub(aim[:], pb_s[:], pa_s[:])
        pc_s, pc_c = angle_products(u, z, RR_ADD, 0.5, M3)       # -M3 snC, M3 cC
        pd_s, pd_c = angle_products(v, w, RR_SUB, 0.5, M4)       # -M4 snD, M4 cD
        bre = keep.tile([P, W], AT, tag="bre", name="bre")
        nc.vector.tensor_add(bre[:], pc_c[:], pd_c[:])
        bim = keep.tile([P, W], AT, tag="bim", name="bim")
        nc.vector.tensor_add(bim[:], pc_s[:], pd_s[:])

        # ---- Breit-Wigner weights w_r = coef_r/(m0^2 - m^2 - i m0 g0) ----
        msq = chain(); nc.vector.tensor_mul(msq[:], t_in["m"][:], t_in["m"][:])
        x = chain()
        for r in range(R):
            nc.vector.tensor_scalar(x[:, _rs(r)], msq[:, _rs(r)], -1.0, m0sq[r],
                                    ALU.mult, ALU.add)
        xsq = chain(); nc.vector.tensor_mul(xsq[:], x[:], x[:])
        den = chain()
        for r in range(R):
            nc.vector.tensor_scalar(den[:, _rs(r)], xsq[:, _rs(r)], ysq[r], None, ALU.add)
        rc = chain()
        nc.vector.reciprocal_approx_fast(out=rc[:], in_=den[:])
        wp1 = chain(); wp2 = chain()
        for r in range(R):
            nc.vector.tensor_scalar(wp1[:, _rs(r)], x[:, _rs(r)], cR[r], k1[r], ALU.mult, ALU.add)
            nc.vector.tensor_scalar(wp2[:, _rs(r)], x[:, _rs(r)], cI[r], k2[r], ALU.mult, ALU.add)
        wre = keep.tile([P, W], F32, tag="wre", name="wre")
        wim = keep.tile([P, W], F32, tag="wim", name="wim")
        nc.vector.tensor_mul(wre[:], wp1[:], rc[:])
        nc.vector.tensor_mul(wim[:], wp2[:], rc[:])
        if BF16_ALGEBRA:
            wreh = keep.tile([P, W], BF16, tag="wreh", name="wreh")
            wimh = keep.tile([P, W], BF16, tag="wimh", name="wimh")
            nc.vector.tensor_copy(wreh[:], wre[:])
            nc.vector.tensor_copy(wimh[:], wim[:])
        else:
            wreh, wimh = wre, wim

        # diag = sum_r |w_r|^2 (fp32)
        d1 = chain(); nc.vector.tensor_mul(d1[:], wre[:], wre[:])
        d2 = chain(); nc.vector.tensor_mul(d2[:], wim[:], wim[:])
        dall = chain(); nc.vector.tensor_add(dall[:], d1[:], d2[:])
        dh = rtile("dh", 1, [P, 2 * E])
        nc.vector.tensor_add(dh[:], dall[:, 0:2*E], dall[:, 2*E:4*E])
        acc = keep.tile([P, E], F32, tag="acc", name="acc")
        dg = small()
        nc.vector.tensor_add(dg[:], dh[:, 0:E], dh[:, E:2*E])
        nc.vector.tensor_scalar(acc[:], dg[:], 7.0, None, ALU.mult)

        # ---- pair interference terms, grouped by r-shift ----
        for sig in (1, 2, 3):
            n = (R - sig) * E
            L = slice(0, n)
            Rr = slice(sig * E, sig * E + n)

            def tmul(a, b):
                o = prot(n, AT)
                nc.vector.tensor_mul(o[:], a[:, L], b[:, Rr])
                return o

            p1 = tmul(are, are); p2 = tmul(aim, aim)
            p3 = tmul(bre, bre); p4 = tmul(bim, bim)
            sa_ = prot(n, AT); nc.vector.tensor_add(sa_[:], p1[:], p2[:])
            sb_ = prot(n, AT); nc.vector.tensor_add(sb_[:], p3[:], p4[:])
            avr = prot(n, AT); nc.vector.tensor_add(avr[:], sa_[:], sb_[:])

            q1 = tmul(are, aim); q2 = tmul(aim, are)
            q3 = tmul(bim, bre); q4 = tmul(bre, bim)
            ia = prot(n, AT); nc.vector.tensor_sub(ia[:], q1[:], q2[:])
            ib = prot(n, AT); nc.vector.tensor_sub(ib[:], q3[:], q4[:])
            avi = prot(n, AT); nc.vector.tensor_add(avi[:], ia[:], ib[:])

            chis = prot(n, F32)
            nc.vector._custom_dve(CHI, out=chis[:], in0=avr[:], in1=avi[:],
                                  s0=20.0, s1=4.0, imm2=-6.0)

            g1 = prot(n, BF16); nc.vector.tensor_mul(g1[:], wreh[:, L], wreh[:, Rr])
            g2 = prot(n, BF16); nc.vector.tensor_mul(g2[:], wimh[:, L], wimh[:, Rr])
            gw = prot(n, F32); nc.vector.tensor_add(gw[:], g1[:], g2[:])

            term = prot(n, F32)
            nc.vector.tensor_mul(term[:], chis[:], gw[:])
            for blk in range(R - sig):
                nc.vector.tensor_add(acc[:], acc[:], term[:, blk*E:(blk+1)*E])

        nc.sync.dma_start(out_ap.rearrange("(p e) -> p e", p=P, e=E), acc[:])

    nc.compile()
    return nc


_CACHE = {}


def _get_nc(m0, g0, coef_r, coef_i):
    key = (m0.tobytes(), g0.tobytes(), coef_r.tobytes(), coef_i.tobytes())
    if key not in _CACHE:
        _CACHE[key] = _build(m0, g0, coef_r, coef_i)
    return _CACHE[key]


def kernel(alpha1, beta1, gamma1, alpha2, beta2, gamma2, m, m0, g0,
           coef_r, coef_i, _want_trace=False):
    nc = _get_nc(np.asarray(m0, np.float32), np.asarray(g0, np.float32),
                 np.asarray(coef_r, np.float32), np.asarray(coef_i, np.float32))
    full = {"alpha1": alpha1, "beta1": beta1, "gamma1": gamma1,
            "alpha2": alpha2, "beta2": beta2, "gamma2": gamma2, "m": m}
    in_maps = []
    for i in range(N_CORES):
        sl = slice(i * N_CORE, (i + 1) * N_CORE)
        in_maps.append({k: np.ascontiguousarray(np.asarray(v, np.float32)[:, sl])
                        for k, v in full.items()})
    res = run_bass_kernel_spmd(nc, in_maps, core_ids=list(range(N_CORES)),
                               trace=_want_trace)
    out = np.concatenate([res.results[i]["out"] for i in range(N_CORES)])
    if _want_trace:
        kernel._last_result = res
    return out.astype(np.float32)


# revision 7
# speedup vs baseline: 1.4956x; 1.0158x over previous
"""Trainium2 Bass kernel for nn_AllAmplitude (helicity-amplitude intensity).

Math: the reference contracts two spin-1 Wigner-D matrices per (resonance,
event) with a Breit-Wigner weight and sums |amp|^2 over external helicities.
Because D1 @ D2 = D^1(U1 U2) for the SU(2) elements U1, U2 of the two
rotations, and sum_{a,dlt} mult_dlt M_r M*_r' = 2 tr D(V) + D(V)[0,0] with
V = U_r'^dag U_r, the whole intensity collapses to

  I = 7 sum_r |w_r|^2 + sum_{r<r'} 2 Re(w_r w*_r') (10 Re(av)^2 + 2 Im(av)^2 - 3)

with av = conj(a_r') a_r + b_r' conj(b_r), (a,b) the Cayley-Klein parameters
of the composed rotation, w_r the complex BW weight.  Per event this is ~200
flops instead of the reference's ~3000.

Sharding: pure data parallelism; the event axis N=262144 is split across the
8 NeuronCores (32768 events each, laid out as [128 partitions x 256 events],
with the R=4 resonance slices side by side in the free axis -> [128, 1024]
working tiles).

Implementation notes:
 - sin/cos of the four composite half-angles via fractional-turn range
   reduction (round-to-nearest through the 1.5*2^23 float trick) feeding the
   ScalarE Sin spline; the whole reduction is one fused custom-DVE op.
 - chi = 20 Re^2 + 4 Im^2 - 6 is a second fused custom-DVE op.
 - the bounded amplitude algebra runs in bf16 (2x DVE tensor_tensor rate);
   angles and Breit-Wigner stay fp32.
"""

import numpy as np
from contextlib import ExitStack

import concourse.bass as bass
import concourse.tile as tile
from concourse import bacc, mybir
from concourse.bass_utils import run_bass_kernel_spmd

F32 = mybir.dt.float32
BF16 = mybir.dt.bfloat16
FP16 = mybir.dt.float16
ALU = mybir.AluOpType
ACTF = mybir.ActivationFunctionType

R = 4
N_TOTAL = 262144
N_CORES = 8
N_CORE = N_TOTAL // N_CORES     # 32768 events per core
P = 128                         # SBUF partitions
E = N_CORE // P                 # 256 events per partition per resonance
W = R * E                       # 1024 free-dim of a full working tile

MAGIC = float(np.float32(1.5 * 2.0**23))   # round-to-nearest-int bias trick
INV4PI = float(np.float32(1.0 / (4.0 * np.pi)))
TWOPI = float(np.float32(2.0 * np.pi))
HALFPI = float(np.float32(np.pi / 2.0))

BF16_ALGEBRA = True   # bounded amplitude algebra in bf16 (2x DVE TT rate)

INPUT_NAMES = ("alpha1", "beta1", "gamma1", "alpha2", "beta2", "gamma2", "m")


# ---------------------------------------------------------------------------
# custom fused DVE ops
# ---------------------------------------------------------------------------

def _register_custom_ops():
    import concourse.dve_ops as dve_ops
    from concourse.dve_spec import Spec, Src0, Src1, C0, C1, C2, sq, lower, _has_src1
    from concourse.dve_uop import DveOpSpec
    from concourse.dve_ops import DveOp

    if any(op.name == "ANT_RANGE_RED_ADD" for op in dve_ops.OPS):
        return {op.name: op for op in dve_ops.OPS}

    def make_op(name, spec):
        shas = {}
        for ver in ("v3", "v4"):
            uops = lower(spec, ver=ver)
            shas[ver] = DveOpSpec(name=name, opcode=31, uops=uops,
                                  rd1_en=_has_src1(spec)).sha(ver)
        return DveOp(name, spec, subdim=False, uops_sha=shas)

    def _rr_ref(sgn):
        def ref(in0, in1, s0, s1, imm2):
            t = ((in0 + sgn * in1) * s0 + s1).astype(np.float32)
            r = ((t + imm2).astype(np.float32) - imm2).astype(np.float32)
            return (t - r).astype(np.float32)
        return ref

    u = (Src0 + Src1) * C0 + C1
    rr_add = make_op("ANT_RANGE_RED_ADD",
                     Spec(body=u - ((u + C2) - C2), reference=_rr_ref(1.0)))
    u2 = (Src0 - Src1) * C0 + C1
    rr_sub = make_op("ANT_RANGE_RED_SUB",
                     Spec(body=u2 - ((u2 + C2) - C2), reference=_rr_ref(-1.0)))
    chi = make_op("ANT_CHI", Spec(
        body=sq(Src0) * C0 + sq(Src1) * C1 + C2,
        reference=lambda in0, in1, s0, s1, imm2:
            (in0 * in0 * s0 + in1 * in1 * s1 + imm2).astype(np.float32)))
    den = make_op("ANT_DEN", Spec(
        body=sq(C0 - Src0) + C1,
        reference=lambda in0, in1, s0, s1, imm2:
            ((s0 - in0) * (s0 - in0) + s1).astype(np.float32)))

    for op in (rr_add, rr_sub, chi, den):
        dve_ops.OPS.append(op)
        dve_ops._SUB_OPCODE_FOR_NAME[op.name] = (
            dve_ops._CUSTOM_DVE_ROW_BASE + len(dve_ops.OPS) - 1)
        dve_ops.CUSTOM_DVE_SPECS[op.name] = op.spec
    assert max(dve_ops._SUB_OPCODE_FOR_NAME.values()) < 0x20
    return {op.name: op for op in dve_ops.OPS}


def _rs(r):
    return slice(r * E, (r + 1) * E)


def _build(m0, g0, coef_r, coef_i):
    """Build + compile the single-core graph (SPMD across 8 cores)."""
    OPS = _register_custom_ops()
    RR_ADD, RR_SUB, CHI, DEN = (OPS["ANT_RANGE_RED_ADD"], OPS["ANT_RANGE_RED_SUB"],
                                OPS["ANT_CHI"], OPS["ANT_DEN"])
    AT = FP16 if BF16_ALGEBRA else F32   # bounded-amplitude algebra dtype

    nc = bacc.Bacc("TRN2", target_bir_lowering=False, debug=False,
                   num_devices=N_CORES)

    ins = {k: nc.dram_tensor(k, (R, N_CORE), F32, kind="ExternalInput").ap()
           for k in INPUT_NAMES}
    out_ap = nc.dram_tensor("out", (N_CORE,), F32, kind="ExternalOutput").ap()

    # per-resonance scalar constants (folded into instructions)
    m0 = m0.astype(np.float64); g0 = g0.astype(np.float64)
    f32 = np.float32
    cR = [float(f32(coef_r[r] * np.cos(coef_i[r]))) for r in range(R)]
    cI = [float(f32(coef_r[r] * np.sin(coef_i[r]))) for r in range(R)]
    m0sq = [float(f32(m0[r] * m0[r])) for r in range(R)]
    y = [float(f32(m0[r] * g0[r])) for r in range(R)]
    ysq = [float(f32(f32(y[r]) * f32(y[r]))) for r in range(R)]
    k1 = [float(f32(-f32(cI[r]) * f32(y[r]))) for r in range(R)]
    k2 = [float(f32(f32(cR[r]) * f32(y[r]))) for r in range(R)]

    with tile.TileContext(nc) as tc, ExitStack() as ctx:
        pin = ctx.enter_context(tc.tile_pool(name="pin", bufs=1))
        keep = ctx.enter_context(tc.tile_pool(name="keep", bufs=1))
        rot = ctx.enter_context(tc.tile_pool(name="rot", bufs=1))
        cnt = [0]

        def rtile(tag, bufs, shape=None, dt=F32):
            cnt[0] += 1
            return rot.tile(shape or [P, W], dt, tag=tag, bufs=bufs,
                            name=f"t{cnt[0]}")

        base = lambda: rtile("base", 4)
        chain = lambda: rtile("chain", 6)
        trig = lambda: rtile("trig", 12, dt=AT)
        pq = lambda: rtile("pq", 5, dt=AT)
        prot = lambda n, dt=F32: rtile("prot", 10, [P, n], dt=dt)
        small = lambda: rtile("small", 4, [P, E])

        # ---- DMA inputs (compute-critical angle tensors first) ----
        t_in = {}
        for k in ("alpha1", "alpha2", "gamma1", "gamma2", "beta1", "beta2", "m"):
            t = pin.tile([P, W], F32, tag=f"in_{k}", name=f"in_{k}")
            for r in range(R):
                nc.sync.dma_start(t[:, _rs(r)],
                                  ins[k][r].rearrange("(p e) -> p e", p=P, e=E))
            t_in[k] = t

        pi2 = keep.tile([P, 1], F32, tag="pi2", name="pi2")
        nc.vector.memset(pi2[:], HALFPI)

        # ---- beta half-angle trig + Wigner-d magnitude products ----
        cb1 = trig(); nc.scalar.activation(cb1[:], t_in["beta1"][:], ACTF.Sin, scale=0.5, bias=pi2[:])
        sb1 = trig(); nc.scalar.activation(sb1[:], t_in["beta1"][:], ACTF.Sin, scale=0.5)
        cb2 = trig(); nc.scalar.activation(cb2[:], t_in["beta2"][:], ACTF.Sin, scale=0.5, bias=pi2[:])
        sb2 = trig(); nc.scalar.activation(sb2[:], t_in["beta2"][:], ACTF.Sin, scale=0.5)
        M1 = trig(); nc.vector.tensor_mul(M1[:], cb1[:], cb2[:])
        M2 = trig(); nc.vector.tensor_mul(M2[:], sb1[:], sb2[:])
        M3 = trig(); nc.vector.tensor_mul(M3[:], cb1[:], sb2[:])
        M4 = trig(); nc.vector.tensor_mul(M4[:], sb1[:], cb2[:])

        # ---- composite half-angles ----
        # A=(a1+a2)+(g1+g2) B=(a1-a2)-(g1-g2) C=(a1+a2)+(g1-g2) D=(a1-a2)-(g1+g2)
        u = base(); nc.vector.tensor_add(u[:], t_in["alpha1"][:], t_in["alpha2"][:])
        v = base(); nc.vector.tensor_sub(v[:], t_in["alpha1"][:], t_in["alpha2"][:])
        w = base(); nc.vector.tensor_add(w[:], t_in["gamma1"][:], t_in["gamma2"][:])
        z = base(); nc.vector.tensor_sub(z[:], t_in["gamma1"][:], t_in["gamma2"][:])

        # Per half-angle X/2: f = frac-reduce(X/(4pi)+off) in [-.5,.5] in one
        # fused DVE op, then Sin(2pi f) on ScalarE, then multiply by the M
        # magnitude.  sin off 0.0 (A,B) / 0.5 (C,D -> negated, cancels in av);
        # cos off 0.25 (positive).
        def angle_products(Xa, Xb, rrop, s_off, M):
            outs = []
            for off in (s_off, 0.25):
                f = chain()
                nc.vector._custom_dve(rrop, out=f[:], in0=Xa[:], in1=Xb[:],
                                      s0=INV4PI, s1=off, imm2=MAGIC)
                sc = trig()
                nc.scalar.activation(sc[:], f[:], ACTF.Sin, scale=TWOPI)
                prod = pq()
                nc.vector.tensor_mul(prod[:], M[:], sc[:])
                outs.append(prod)
            return outs  # [M*sin-ish, M*cos]

        pa_s, pa_c = angle_products(u, w, RR_ADD, 0.0, M1)       # M1 snA, M1 cA
        pb_s, pb_c = angle_products(v, z, RR_SUB, 0.0, M2)       # M2 snB, M2 cB
        are = keep.tile([P, W], AT, tag="are", name="are")
        nc.vector.tensor_sub(are[:], pa_c[:], pb_c[:])
        aim = keep.tile([P, W], AT, tag="aim", name="aim")
        nc.vector.tensor_sub(aim[:], pb_s[:], pa_s[:])
        pc_s, pc_c = angle_products(u, z, RR_ADD, 0.5, M3)       # -M3 snC, M3 cC
        pd_s, pd_c = angle_products(v, w, RR_SUB, 0.5, M4)       # -M4 snD, M4 cD
        bre = keep.tile([P, W], AT, tag="bre", name="bre")
        nc.vector.tensor_add(bre[:], pc_c[:], pd_c[:])
        bim = keep.tile([P, W], AT, tag="bim", name="bim")
        nc.vector.tensor_add(bim[:], pc_s[:], pd_s[:])

        # ---- Breit-Wigner weights w_r = coef_r/(m0^2 - m^2 - i m0 g0) ----
        msq = chain(); nc.vector.tensor_mul(msq[:], t_in["m"][:], t_in["m"][:])
        den = chain()
        for r in range(R):
            nc.vector._custom_dve(DEN, out=den[:, _rs(r)], in0=msq[:, _rs(r)],
                                  s0=m0sq[r], s1=ysq[r])
        rc = chain()
        nc.vector.reciprocal_approx_fast(out=rc[:], in_=den[:])
        # w numerators straight from m^2:  x = m0^2 - msq
        #   wp1 = x*cR + k1 = -cR*msq + (cR*m0^2 + k1)
        wp1 = chain(); wp2 = chain()
        for r in range(R):
            nc.vector.tensor_scalar(wp1[:, _rs(r)], msq[:, _rs(r)], -cR[r],
                                    float(np.float32(cR[r]*m0sq[r] + k1[r])), ALU.mult, ALU.add)
            nc.vector.tensor_scalar(wp2[:, _rs(r)], msq[:, _rs(r)], -cI[r],
                                    float(np.float32(cI[r]*m0sq[r] + k2[r])), ALU.mult, ALU.add)
        wre = keep.tile([P, W], F32, tag="wre", name="wre")
        wim = keep.tile([P, W], F32, tag="wim", name="wim")
        nc.vector.tensor_mul(wre[:], wp1[:], rc[:])
        nc.vector.tensor_mul(wim[:], wp2[:], rc[:])
        if BF16_ALGEBRA:
            wreh = keep.tile([P, W], BF16, tag="wreh", name="wreh")
            wimh = keep.tile([P, W], BF16, tag="wimh", name="wimh")
            nc.vector.tensor_copy(wreh[:], wre[:])
            nc.vector.tensor_copy(wimh[:], wim[:])
        else:
            wreh, wimh = wre, wim

        # diag = sum_r |w_r|^2 (fp32)
        d1 = chain(); nc.vector.tensor_mul(d1[:], wre[:], wre[:])
        d2 = chain(); nc.vector.tensor_mul(d2[:], wim[:], wim[:])
        dall = chain(); nc.vector.tensor_add(dall[:], d1[:], d2[:])
        dh = rtile("dh", 1, [P, 2 * E])
        nc.vector.tensor_add(dh[:], dall[:, 0:2*E], dall[:, 2*E:4*E])
        acc = keep.tile([P, E], F32, tag="acc", name="acc")
        dg = small()
        nc.vector.tensor_add(dg[:], dh[:, 0:E], dh[:, E:2*E])
        nc.vector.tensor_scalar(acc[:], dg[:], 7.0, None, ALU.mult)

        # ---- pair interference terms, grouped by r-shift ----
        for sig in (1, 2, 3):
            n = (R - sig) * E
            L = slice(0, n)
            Rr = slice(sig * E, sig * E + n)

            def tmul(a, b):
                o = prot(n, AT)
                nc.vector.tensor_mul(o[:], a[:, L], b[:, Rr])
                return o

            p1 = tmul(are, are); p2 = tmul(aim, aim)
            p3 = tmul(bre, bre); p4 = tmul(bim, bim)
            sa_ = prot(n, AT); nc.vector.tensor_add(sa_[:], p1[:], p2[:])
            sb_ = prot(n, AT); nc.vector.tensor_add(sb_[:], p3[:], p4[:])
            avr = prot(n, AT); nc.vector.tensor_add(avr[:], sa_[:], sb_[:])

            q1 = tmul(are, aim); q2 = tmul(aim, are)
            q3 = tmul(bim, bre); q4 = tmul(bre, bim)
            ia = prot(n, AT); nc.vector.tensor_sub(ia[:], q1[:], q2[:])
            ib = prot(n, AT); nc.vector.tensor_sub(ib[:], q3[:], q4[:])
            avi = prot(n, AT); nc.vector.tensor_add(avi[:], ia[:], ib[:])

            chis = prot(n, F32)
            nc.vector._custom_dve(CHI, out=chis[:], in0=avr[:], in1=avi[:],
                                  s0=20.0, s1=4.0, imm2=-6.0)

            g1 = prot(n, BF16); nc.vector.tensor_mul(g1[:], wreh[:, L], wreh[:, Rr])
            g2 = prot(n, BF16); nc.vector.tensor_mul(g2[:], wimh[:, L], wimh[:, Rr])
            gw = prot(n, F32); nc.vector.tensor_add(gw[:], g1[:], g2[:])

            term = prot(n, F32)
            nc.vector.tensor_mul(term[:], chis[:], gw[:])
            for blk in range(R - sig):
                nc.vector.tensor_add(acc[:], acc[:], term[:, blk*E:(blk+1)*E])

        nc.sync.dma_start(out_ap.rearrange("(p e) -> p e", p=P, e=E), acc[:])

    nc.compile()
    return nc


_CACHE = {}


def _get_nc(m0, g0, coef_r, coef_i):
    key = (m0.tobytes(), g0.tobytes(), coef_r.tobytes(), coef_i.tobytes())
    if key not in _CACHE:
        _CACHE[key] = _build(m0, g0, coef_r, coef_i)
    return _CACHE[key]


def kernel(alpha1, beta1, gamma1, alpha2, beta2, gamma2, m, m0, g0,
           coef_r, coef_i, _want_trace=False):
    nc = _get_nc(np.asarray(m0, np.float32), np.asarray(g0, np.float32),
                 np.asarray(coef_r, np.float32), np.asarray(coef_i, np.float32))
    full = {"alpha1": alpha1, "beta1": beta1, "gamma1": gamma1,
            "alpha2": alpha2, "beta2": beta2, "gamma2": gamma2, "m": m}
    in_maps = []
    for i in range(N_CORES):
        sl = slice(i * N_CORE, (i + 1) * N_CORE)
        in_maps.append({k: np.ascontiguousarray(np.asarray(v, np.float32)[:, sl])
                        for k, v in full.items()})
    res = run_bass_kernel_spmd(nc, in_maps, core_ids=list(range(N_CORES)),
                               trace=_want_trace)
    out = np.concatenate([res.results[i]["out"] for i in range(N_CORES)])
    if _want_trace:
        kernel._last_result = res
    return out.astype(np.float32)


# revision 9
# speedup vs baseline: 1.7219x; 1.1513x over previous
"""Trainium2 Bass kernel for nn_AllAmplitude (helicity-amplitude intensity).

Math: the reference contracts two spin-1 Wigner-D matrices per (resonance,
event) with a Breit-Wigner weight and sums |amp|^2 over external helicities.
Because D1 @ D2 = D^1(U1 U2) for the SU(2) elements U1, U2 of the two
rotations, and sum_{a,dlt} mult_dlt M_r M*_r' = 2 tr D(V) + D(V)[0,0] with
V = U_r'^dag U_r, the whole intensity collapses to

  I = 7 sum_r |w_r|^2 + sum_{r<r'} 2 Re(w_r w*_r') (10 Re(av)^2 + 2 Im(av)^2 - 3)

with av = conj(a_r') a_r + b_r' conj(b_r), (a,b) the Cayley-Klein parameters
of the composed rotation, w_r the complex BW weight.  Per event this is ~200
flops instead of the reference's ~3000.

Sharding: pure data parallelism; the event axis N=262144 is split across the
8 NeuronCores (32768 events each, laid out as [128 partitions x 256 events],
with the R=4 resonance slices side by side in the free axis -> [128, 1024]
working tiles).

Implementation notes:
 - sin/cos of the four composite half-angles via fractional-turn range
   reduction (round-to-nearest through the 1.5*2^23 float trick) feeding the
   ScalarE Sin spline; the whole reduction is one fused custom-DVE op.
 - chi = 20 Re^2 + 4 Im^2 - 6 is a second fused custom-DVE op.
 - the bounded amplitude algebra runs in bf16 (2x DVE tensor_tensor rate);
   angles and Breit-Wigner stay fp32.
"""

import numpy as np
from contextlib import ExitStack

import concourse.bass as bass
import concourse.tile as tile
from concourse import bacc, mybir
from concourse.bass_utils import run_bass_kernel_spmd

F32 = mybir.dt.float32
BF16 = mybir.dt.bfloat16
FP16 = mybir.dt.float16
ALU = mybir.AluOpType
ACTF = mybir.ActivationFunctionType

R = 4
N_TOTAL = 262144
N_CORES = 8
N_CORE = N_TOTAL // N_CORES     # 32768 events per core
P = 128                         # SBUF partitions
E = N_CORE // P                 # 256 events per partition per resonance
W = R * E                       # 1024 free-dim of a full working tile

MAGIC = float(np.float32(1.5 * 2.0**23))   # round-to-nearest-int bias trick
INV4PI = float(np.float32(1.0 / (4.0 * np.pi)))
TWOPI = float(np.float32(2.0 * np.pi))
HALFPI = float(np.float32(np.pi / 2.0))

BF16_ALGEBRA = True   # bounded amplitude algebra in bf16 (2x DVE TT rate)

INPUT_NAMES = ("alpha1", "beta1", "gamma1", "alpha2", "beta2", "gamma2", "m")


# ---------------------------------------------------------------------------
# custom fused DVE ops
# ---------------------------------------------------------------------------

def _register_custom_ops():
    import concourse.dve_ops as dve_ops
    from concourse.dve_spec import Spec, Src0, Src1, C0, C1, C2, sq, lower, _has_src1
    from concourse.dve_uop import DveOpSpec
    from concourse.dve_ops import DveOp

    if any(op.name == "ANT_RANGE_RED_ADD" for op in dve_ops.OPS):
        return {op.name: op for op in dve_ops.OPS}

    def make_op(name, spec):
        shas = {}
        for ver in ("v3", "v4"):
            uops = lower(spec, ver=ver)
            shas[ver] = DveOpSpec(name=name, opcode=31, uops=uops,
                                  rd1_en=_has_src1(spec)).sha(ver)
        return DveOp(name, spec, subdim=False, uops_sha=shas)

    def _rr_ref(sgn):
        def ref(in0, in1, s0, s1, imm2):
            t = ((in0 + sgn * in1) * s0 + s1).astype(np.float32)
            r = ((t + imm2).astype(np.float32) - imm2).astype(np.float32)
            return (t - r).astype(np.float32)
        return ref

    u = (Src0 + Src1) * C0 + C1
    rr_add = make_op("ANT_RANGE_RED_ADD",
                     Spec(body=u - ((u + C2) - C2), reference=_rr_ref(1.0)))
    u2 = (Src0 - Src1) * C0 + C1
    rr_sub = make_op("ANT_RANGE_RED_SUB",
                     Spec(body=u2 - ((u2 + C2) - C2), reference=_rr_ref(-1.0)))
    chi = make_op("ANT_CHI", Spec(
        body=sq(Src0) * C0 + sq(Src1) * C1 + C2,
        reference=lambda in0, in1, s0, s1, imm2:
            (in0 * in0 * s0 + in1 * in1 * s1 + imm2).astype(np.float32)))
    den = make_op("ANT_DEN", Spec(
        body=sq(C0 - Src0) + C1,
        reference=lambda in0, in1, s0, s1, imm2:
            ((s0 - in0) * (s0 - in0) + s1).astype(np.float32)))

    for op in (rr_add, rr_sub, chi, den):
        dve_ops.OPS.append(op)
        dve_ops._SUB_OPCODE_FOR_NAME[op.name] = (
            dve_ops._CUSTOM_DVE_ROW_BASE + len(dve_ops.OPS) - 1)
        dve_ops.CUSTOM_DVE_SPECS[op.name] = op.spec
    assert max(dve_ops._SUB_OPCODE_FOR_NAME.values()) < 0x20
    return {op.name: op for op in dve_ops.OPS}


def _rs(r):
    return slice(r * E, (r + 1) * E)


def _build(m0, g0, coef_r, coef_i):
    """Build + compile the single-core graph (SPMD across 8 cores)."""
    OPS = _register_custom_ops()
    RR_ADD, RR_SUB, CHI, DEN = (OPS["ANT_RANGE_RED_ADD"], OPS["ANT_RANGE_RED_SUB"],
                                OPS["ANT_CHI"], OPS["ANT_DEN"])
    AT = FP16 if BF16_ALGEBRA else F32   # bounded-amplitude algebra dtype

    nc = bacc.Bacc("TRN2", target_bir_lowering=False, debug=False,
                   num_devices=N_CORES)

    ins = {k: nc.dram_tensor(k, (R, N_CORE), F32, kind="ExternalInput").ap()
           for k in INPUT_NAMES}
    out_ap = nc.dram_tensor("out", (N_CORE,), F32, kind="ExternalOutput").ap()

    # per-resonance scalar constants (folded into instructions)
    m0 = m0.astype(np.float64); g0 = g0.astype(np.float64)
    f32 = np.float32
    cR = [float(f32(coef_r[r] * np.cos(coef_i[r]))) for r in range(R)]
    cI = [float(f32(coef_r[r] * np.sin(coef_i[r]))) for r in range(R)]
    m0sq = [float(f32(m0[r] * m0[r])) for r in range(R)]
    y = [float(f32(m0[r] * g0[r])) for r in range(R)]
    ysq = [float(f32(f32(y[r]) * f32(y[r]))) for r in range(R)]
    k1 = [float(f32(-f32(cI[r]) * f32(y[r]))) for r in range(R)]
    k2 = [float(f32(f32(cR[r]) * f32(y[r]))) for r in range(R)]

    with tile.TileContext(nc) as tc, ExitStack() as ctx:
        pin = ctx.enter_context(tc.tile_pool(name="pin", bufs=1))
        keep = ctx.enter_context(tc.tile_pool(name="keep", bufs=1))
        rot = ctx.enter_context(tc.tile_pool(name="rot", bufs=1))
        cnt = [0]

        def rtile(tag, bufs, shape=None, dt=F32):
            cnt[0] += 1
            return rot.tile(shape or [P, W], dt, tag=tag, bufs=bufs,
                            name=f"t{cnt[0]}")

        base = lambda: rtile("base", 4)
        chain = lambda: rtile("chain", 6)
        trig = lambda: rtile("trig", 12, dt=AT)
        pq = lambda: rtile("pq", 5, dt=AT)
        prot = lambda n, dt=F32: rtile("prot", 10, [P, n], dt=dt)
        small = lambda: rtile("small", 4, [P, E])

        # ---- DMA inputs (compute-critical angle tensors first, r-interleaved
        # so the first r-slices of each operand pair land ASAP) ----
        t_in = {k: pin.tile([P, W], F32, tag=f"in_{k}", name=f"in_{k}")
                for k in INPUT_NAMES}

        def dma_in(k, r):
            nc.sync.dma_start(t_in[k][:, _rs(r)],
                              ins[k][r].rearrange("(p e) -> p e", p=P, e=E))
        for r in range(R):
            dma_in("alpha1", r); dma_in("alpha2", r)
        for r in range(R):
            dma_in("gamma1", r); dma_in("gamma2", r)
        for r in range(R):
            dma_in("beta1", r); dma_in("beta2", r)
        for r in range(R):
            dma_in("m", r)

        pi2 = keep.tile([P, 1], F32, tag="pi2", name="pi2")
        nc.vector.memset(pi2[:], HALFPI)

        # ---- beta half-angle trig + Wigner-d magnitude products ----
        cb1 = trig(); nc.scalar.activation(cb1[:], t_in["beta1"][:], ACTF.Sin, scale=0.5, bias=pi2[:])
        sb1 = trig(); nc.scalar.activation(sb1[:], t_in["beta1"][:], ACTF.Sin, scale=0.5)
        cb2 = trig(); nc.scalar.activation(cb2[:], t_in["beta2"][:], ACTF.Sin, scale=0.5, bias=pi2[:])
        sb2 = trig(); nc.scalar.activation(sb2[:], t_in["beta2"][:], ACTF.Sin, scale=0.5)
        M1 = trig(); nc.vector.tensor_mul(M1[:], cb1[:], cb2[:])
        M2 = trig(); nc.vector.tensor_mul(M2[:], sb1[:], sb2[:])
        M3 = trig(); nc.vector.tensor_mul(M3[:], cb1[:], sb2[:])
        M4 = trig(); nc.vector.tensor_mul(M4[:], sb1[:], cb2[:])

        # ---- composite half-angles ----
        # A=(a1+a2)+(g1+g2) B=(a1-a2)-(g1-g2) C=(a1+a2)+(g1-g2) D=(a1-a2)-(g1+g2)
        u = base(); v = base(); w = base(); z = base()
        for r in range(R):
            s = _rs(r)
            nc.vector.tensor_add(u[:, s], t_in["alpha1"][:, s], t_in["alpha2"][:, s])
            nc.vector.tensor_sub(v[:, s], t_in["alpha1"][:, s], t_in["alpha2"][:, s])
        for r in range(R):
            s = _rs(r)
            nc.vector.tensor_add(w[:, s], t_in["gamma1"][:, s], t_in["gamma2"][:, s])
            nc.vector.tensor_sub(z[:, s], t_in["gamma1"][:, s], t_in["gamma2"][:, s])

        # Per half-angle X/2: f = frac-reduce(X/(4pi)+off) in [-.5,.5] in one
        # fused DVE op, then Sin(2pi f) on ScalarE, then multiply by the M
        # magnitude.  sin off 0.0 (A,B) / 0.5 (C,D -> negated, cancels in av);
        # cos off 0.25 (positive).
        def angle_products(Xa, Xb, rrop, s_off, M):
            outs = []
            for off in (s_off, 0.25):
                f = chain()
                nc.vector._custom_dve(rrop, out=f[:], in0=Xa[:], in1=Xb[:],
                                      s0=INV4PI, s1=off, imm2=MAGIC)
                sc = trig()
                nc.scalar.activation(sc[:], f[:], ACTF.Sin, scale=TWOPI)
                prod = pq()
                nc.vector.tensor_mul(prod[:], M[:], sc[:])
                outs.append(prod)
            return outs  # [M*sin-ish, M*cos]

        pa_s, pa_c = angle_products(u, w, RR_ADD, 0.0, M1)       # M1 snA, M1 cA
        pb_s, pb_c = angle_products(v, z, RR_SUB, 0.0, M2)       # M2 snB, M2 cB
        are = keep.tile([P, W], AT, tag="are", name="are")
        nc.vector.tensor_sub(are[:], pa_c[:], pb_c[:])
        aim = keep.tile([P, W], AT, tag="aim", name="aim")
        nc.vector.tensor_sub(aim[:], pb_s[:], pa_s[:])
        pc_s, pc_c = angle_products(u, z, RR_ADD, 0.5, M3)       # -M3 snC, M3 cC
        pd_s, pd_c = angle_products(v, w, RR_SUB, 0.5, M4)       # -M4 snD, M4 cD
        bre = keep.tile([P, W], AT, tag="bre", name="bre")
        nc.vector.tensor_add(bre[:], pc_c[:], pd_c[:])
        bim = keep.tile([P, W], AT, tag="bim", name="bim")
        nc.vector.tensor_add(bim[:], pc_s[:], pd_s[:])

        # ---- Breit-Wigner weights w_r = coef_r/(m0^2 - m^2 - i m0 g0) ----
        msq = chain(); nc.scalar.activation(msq[:], t_in["m"][:], ACTF.Square)
        den = chain()
        for r in range(R):
            nc.vector._custom_dve(DEN, out=den[:, _rs(r)], in0=msq[:, _rs(r)],
                                  s0=m0sq[r], s1=ysq[r])
        rc = chain()
        nc.vector.reciprocal_approx_fast(out=rc[:], in_=den[:])
        # w numerators straight from m^2:  x = m0^2 - msq
        #   wp1 = x*cR + k1 = -cR*msq + (cR*m0^2 + k1)
        wp1 = chain(); wp2 = chain()
        for r in range(R):
            nc.scalar.activation(wp1[:, _rs(r)], msq[:, _rs(r)], ACTF.Copy,
                                 scale=-cR[r],
                                 bias=float(np.float32(cR[r]*m0sq[r] + k1[r])))
            nc.scalar.activation(wp2[:, _rs(r)], msq[:, _rs(r)], ACTF.Copy,
                                 scale=-cI[r],
                                 bias=float(np.float32(cI[r]*m0sq[r] + k2[r])))
        wre = keep.tile([P, W], F32, tag="wre", name="wre")
        wim = keep.tile([P, W], F32, tag="wim", name="wim")
        nc.vector.tensor_mul(wre[:], wp1[:], rc[:])
        nc.vector.tensor_mul(wim[:], wp2[:], rc[:])
        if BF16_ALGEBRA:
            wreh = keep.tile([P, W], BF16, tag="wreh", name="wreh")
            wimh = keep.tile([P, W], BF16, tag="wimh", name="wimh")
            nc.scalar.activation(wreh[:], wre[:], ACTF.Copy)
            nc.scalar.activation(wimh[:], wim[:], ACTF.Copy)
        else:
            wreh, wimh = wre, wim

        # diag = sum_r |w_r|^2 (fp32)
        d1 = chain(); nc.vector.tensor_mul(d1[:], wre[:], wre[:])
        d2 = chain(); nc.vector.tensor_mul(d2[:], wim[:], wim[:])
        dall = chain(); nc.vector.tensor_add(dall[:], d1[:], d2[:])
        dh = rtile("dh", 1, [P, 2 * E])
        nc.vector.tensor_add(dh[:], dall[:, 0:2*E], dall[:, 2*E:4*E])
        acc = keep.tile([P, E], F32, tag="acc", name="acc")
        dg = small()
        nc.vector.tensor_add(dg[:], dh[:, 0:E], dh[:, E:2*E])
        nc.scalar.activation(acc[:], dg[:], ACTF.Copy, scale=7.0)

        # ---- pair interference terms, grouped by r-shift ----
        for sig in (1, 2, 3):
            n = (R - sig) * E
            L = slice(0, n)
            Rr = slice(sig * E, sig * E + n)

            def tmul(a, b):
                o = prot(n, AT)
                nc.vector.tensor_mul(o[:], a[:, L], b[:, Rr])
                return o

            p1 = tmul(are, are); p2 = tmul(aim, aim)
            p3 = tmul(bre, bre); p4 = tmul(bim, bim)
            sa_ = prot(n, AT); nc.vector.tensor_add(sa_[:], p1[:], p2[:])
            sb_ = prot(n, AT); nc.vector.tensor_add(sb_[:], p3[:], p4[:])
            avr = prot(n, AT); nc.vector.tensor_add(avr[:], sa_[:], sb_[:])

            q1 = tmul(are, aim); q2 = tmul(aim, are)
            q3 = tmul(bim, bre); q4 = tmul(bre, bim)
            ia = prot(n, AT); nc.vector.tensor_sub(ia[:], q1[:], q2[:])
            ib = prot(n, AT); nc.vector.tensor_sub(ib[:], q3[:], q4[:])
            avi = prot(n, AT); nc.vector.tensor_add(avi[:], ia[:], ib[:])

            chis = prot(n, F32)
            nc.vector._custom_dve(CHI, out=chis[:], in0=avr[:], in1=avi[:],
                                  s0=20.0, s1=4.0, imm2=-6.0)

            g1 = prot(n, BF16); nc.vector.tensor_mul(g1[:], wreh[:, L], wreh[:, Rr])
            g2 = prot(n, BF16); nc.vector.tensor_mul(g2[:], wimh[:, L], wimh[:, Rr])
            gw = prot(n, F32); nc.vector.tensor_add(gw[:], g1[:], g2[:])

            term = prot(n, F32)
            nc.vector.tensor_mul(term[:], chis[:], gw[:])
            for blk in range(R - sig):
                nc.vector.tensor_add(acc[:], acc[:], term[:, blk*E:(blk+1)*E])

        nc.sync.dma_start(out_ap.rearrange("(p e) -> p e", p=P, e=E), acc[:])

    nc.compile()
    return nc


_CACHE = {}


def _get_nc(m0, g0, coef_r, coef_i):
    key = (m0.tobytes(), g0.tobytes(), coef_r.tobytes(), coef_i.tobytes())
    if key not in _CACHE:
        _CACHE[key] = _build(m0, g0, coef_r, coef_i)
    return _CACHE[key]


def kernel(alpha1, beta1, gamma1, alpha2, beta2, gamma2, m, m0, g0,
           coef_r, coef_i, _want_trace=False):
    nc = _get_nc(np.asarray(m0, np.float32), np.asarray(g0, np.float32),
                 np.asarray(coef_r, np.float32), np.asarray(coef_i, np.float32))
    full = {"alpha1": alpha1, "beta1": beta1, "gamma1": gamma1,
            "alpha2": alpha2, "beta2": beta2, "gamma2": gamma2, "m": m}
    in_maps = []
    for i in range(N_CORES):
        sl = slice(i * N_CORE, (i + 1) * N_CORE)
        in_maps.append({k: np.ascontiguousarray(np.asarray(v, np.float32)[:, sl])
                        for k, v in full.items()})
    res = run_bass_kernel_spmd(nc, in_maps, core_ids=list(range(N_CORES)),
                               trace=_want_trace)
    out = np.concatenate([res.results[i]["out"] for i in range(N_CORES)])
    if _want_trace:
        kernel._last_result = res
    return out.astype(np.float32)
